# revision 17
# baseline (speedup 1.0000x reference)
"""GPTQ group-quantized linear (nn_GPTQLinear) on 8 Trainium2 NeuronCores.

out[b,s,o] = sum_k x[b,s,k] * (qweight[o,k] * scales[o, k//128]) + bias[o]

Full inputs in, full output out.  Sharding (internal): 4-way over batch rows
x 2-way over out_features -> per core M=2048 rows, N=2048 out feats, K=4096.

Per-core kernel (v2 -- PE stream is pure matmuls):
  - qweight int32 [o,k] -> DVE dequant (x group scale bcast along free) ->
    bf16 [o,k] -> SBUF->SBUF DMA-xbar transpose -> resident wT
    [128, K/128, N] bf16 in SBUF.  No PE transposes, no DRAM staging.
  - x fp32 [m,k] -> SWDGE cast DMA (DRAM->SBUF, big row descriptors) ->
    bf16 [m,k] -> SBUF->SBUF DMA-xbar transpose -> xT per 128-row chunk.
  - bf16 matmuls (lhsT = xT slice, rhs = wT slice), fp32 accumulate in PSUM
    over K, bias added by DVE in the epilogue.
  - Startup: first two output chains run in two K-sessions so matmuls start
    after only 1/8th of qweight is dequantized; first 5 x-chunks are swept
    n-group by n-group as dequant completes, so the PE never idles long.
"""

from contextlib import ExitStack

import numpy as np

import concourse.bass as bass
import concourse.bacc as bacc
import concourse.mybir as mybir
import concourse.tile as tile
from concourse import bass_utils
from concourse.masks import make_identity

F32 = mybir.dt.float32
BF16 = mybir.dt.bfloat16
I32 = mybir.dt.int32

P = 128            # partitions = k-tile = quant group size
N_CH = 512         # out-feature chunk (one PSUM bank of fp32)
M_SC = 256         # x rows per block (2 lhsT tiles)

# full problem / sharding constants (hardcoded per harness contract)
B, S, K_FULL, NF = 4, 2048, 4096, 4096
MB_SHARDS, NB_SHARDS = 4, 2
M_CORE, N_CORE = (B * S) // MB_SHARDS, NF // NB_SHARDS
N_CORES = 8


def emit(tc, ctx, o_ap, x_ap, q_ap, s_ap, b_ap):
    nc = tc.nc
    M, K = x_ap.shape
    N = q_ap.shape[0]
    KT = K // P                   # 32 k-tiles
    NCH = N // N_CH               # 4 out-feature chunks
    NSC = M // M_SC               # 8 row blocks of 256
    NOC = N // P                  # 16 o-tiles
    OC_PER_CH = N_CH // P         # 4 o-tiles per out chunk
    MT = M_SC // P                # 2 m-tiles per row block
    KH = K // 2                   # x-cast chunk extent (2048)
    KQ = K // 4                   # dequant quarter-slab extent (1024)
    GQ = KQ // P                  # k-tiles (= groups) per quarter slab (8)

    const = ctx.enter_context(tc.tile_pool(name="const", bufs=1))
    wt_pool = ctx.enter_context(tc.tile_pool(name="wt", bufs=1))
    q_pool = ctx.enter_context(tc.tile_pool(name="qs", bufs=2))
    wdq_pool = ctx.enter_context(tc.tile_pool(name="wdq", bufs=4))
    xbf_pool = ctx.enter_context(tc.tile_pool(name="xbf", bufs=2))
    xt_pool = ctx.enter_context(tc.tile_pool(name="xt", bufs=2))
    out_pool = ctx.enter_context(tc.tile_pool(name="outp", bufs=3))
    ps_pool = ctx.enter_context(tc.tile_pool(name="psmm", bufs=5, space="PSUM"))
    pst_pool = ctx.enter_context(tc.tile_pool(name="pst", bufs=2, space="PSUM"))
    pan_pool = ctx.enter_context(tc.tile_pool(name="pan", bufs=NSC, space="DRAM"))

    # ---- constants ----
    scales_sb = const.tile([P, NOC, KT], F32, tag="scales")
    nc.scalar.dma_start(scales_sb[:], s_ap.rearrange("(oc p) g -> p oc g", p=P))
    bias_sb = const.tile([1, N], BF16, tag="bias")
    nc.gpsimd.dma_start(bias_sb[:], b_ap[None, :])
    ones = const.tile([1, P], BF16, tag="ones")
    nc.vector.memset(ones[:], 1.0)
    identity = const.tile([P, P], BF16, tag="identity")
    make_identity(nc, identity[:])

    # bias broadcast to all 128 partitions via a K=1 matmul
    bias_bc = const.tile([P, N], BF16, tag="bias_bc")
    for n in range(NCH):
        psb = ps_pool.tile([P, N_CH], F32, name="ps_bias", bufs=1)
        nc.tensor.matmul(
            psb[:], ones[:], bias_sb[:, n * N_CH : (n + 1) * N_CH],
            start=True, stop=True,
        )
        nc.vector.tensor_copy(bias_bc[:, n * N_CH : (n + 1) * N_CH], psb[:])

    # resident transposed weights: wt[n][p, kt, o] for out chunk n
    wt = [
        wt_pool.tile([P, KT, N_CH], BF16, tag=f"wt{n}", name=f"wt{n}")
        for n in range(NCH)
    ]

    def dq_load_mult(oc, kq):
        """Dequant o-tile oc k-quarter kq: q load + DVE mult into a wdq buf."""
        qt = q_pool.tile([P, KQ], I32, name="qt")
        nc.scalar.dma_start(qt[:], q_ap[oc * P : (oc + 1) * P, kq * KQ : (kq + 1) * KQ])
        wdq = wdq_pool.tile([P, KQ], BF16, name="wdq")
        nc.vector.tensor_tensor(
            wdq[:].rearrange("p (g i) -> p g i", i=P),
            qt[:].rearrange("p (g i) -> p g i", i=P),
            scales_sb[:, oc, kq * GQ : (kq + 1) * GQ, None].to_broadcast([P, GQ, P]),
            mybir.AluOpType.mult,
        )
        return wdq

    def dq_transpose_copy(oc, kq, wdq):
        """PE transpose (8 k-tiles through one PSUM bank) -> DVE copy to wt."""
        n_ch, oci = divmod(oc, OC_PER_CH)
        pst = pst_pool.tile([P, KQ], BF16, name="pst")
        for j in range(GQ):
            nc.tensor.transpose(
                pst[:, j * P : (j + 1) * P], wdq[:, j * P : (j + 1) * P], identity[:]
            )
        nc.vector.tensor_copy(
            wt[n_ch][:, kq * GQ : (kq + 1) * GQ, oci * P : (oci + 1) * P],
            pst[:].rearrange("p (g i) -> p g i", i=P),
        )

    def dq_group(units):
        """Pipeline a group of dequant units: all mults ahead of all copies."""
        wdqs = [dq_load_mult(oc, kq) for oc, kq in units]
        for (oc, kq), wdq in zip(units, wdqs):
            dq_transpose_copy(oc, kq, wdq)

    xt_tiles = {}

    def x_unit(sc):
        """x rows [sc*256,(sc+1)*256): SWDGE cast to bf16 SBUF, HWDGE stage to
        a DRAM panel, one folded xbar transpose panel -> xT[p, kt, m]."""
        pan = pan_pool.tile([M_SC, K], BF16, name=f"pan{sc}", tag=f"pan{sc}")
        for mt in range(MT):
            m0 = sc * M_SC + mt * P
            for h in range(2):
                xbf = xbf_pool.tile([P, KH], BF16, name="xbf")
                nc.gpsimd.dma_start(xbf[:], x_ap[m0 : m0 + P, h * KH : (h + 1) * KH])
                nc.sync.dma_start(
                    pan[mt * P : (mt + 1) * P, h * KH : (h + 1) * KH], xbf[:]
                )
        xt = xt_pool.tile([P, KT, M_SC], BF16, name="xt")
        nc.sync.dma_start(xt[:], pan[:], transpose=True)
        xt_tiles[sc] = xt
        return xt

    open_chains = {}

    def chain_mm(sc, mt, n, k_lo, k_hi):
        """Emit matmuls k in [k_lo, k_hi) of chain (sc, mt, n); close at K."""
        if k_lo == 0:
            open_chains[(sc, mt, n)] = ps_pool.tile([P, N_CH], F32, name="ps_mm")
        ps = open_chains[(sc, mt, n)]
        xt = xt_tiles[sc]
        for k in range(k_lo, k_hi):
            nc.tensor.matmul(
                ps[:], xt[:, k, mt * P : (mt + 1) * P], wt[n][:, k, :],
                start=(k == 0), stop=(k == KT - 1),
            )
        if k_hi < KT:
            return
        del open_chains[(sc, mt, n)]
        ot = out_pool.tile([P, N_CH], F32, name="ot")
        nc.vector.tensor_tensor(
            ot[:], ps[:], bias_bc[:, n * N_CH : (n + 1) * N_CH],
            mybir.AluOpType.add,
        )
        m0 = sc * M_SC + mt * P
        nc.sync.dma_start(o_ap[m0 : m0 + P, n * N_CH : (n + 1) * N_CH], ot[:])

    def block(sc, n):
        for mt in range(MT):
            chain_mm(sc, mt, n, 0, KT)

    def block_fused(sc):
        """All 4 out chunks of one m-tile per k step: stationary xt tile is
        reused across the 4 rhs chunks."""
        xt = xt_tiles[sc]
        for mt in range(MT):
            pss = [ps_pool.tile([P, N_CH], F32, name="ps_mm") for _ in range(NCH)]
            for k in range(KT):
                for n in range(NCH):
                    nc.tensor.matmul(
                        pss[n][:], xt[:, k, mt * P : (mt + 1) * P], wt[n][:, k, :],
                        start=(k == 0), stop=(k == KT - 1),
                    )
            m0 = sc * M_SC + mt * P
            for n in range(NCH):
                ot = out_pool.tile([P, N_CH], F32, name="ot")
                nc.vector.tensor_tensor(
                    ot[:], pss[n][:], bias_bc[:, n * N_CH : (n + 1) * N_CH],
                    mybir.AluOpType.add,
                )
                nc.sync.dma_start(
                    o_ap[m0 : m0 + P, n * N_CH : (n + 1) * N_CH], ot[:]
                )

    # ---- schedule ----
    x_unit(0)
    x_unit(1)
    # n0 quarter-by-quarter; block-0 chains run in 4 K-sessions
    for kq in range(4):
        dq_group([(oc, kq) for oc in range(0, 4)])
        chain_mm(0, 0, 0, kq * GQ, (kq + 1) * GQ)
        chain_mm(0, 1, 0, kq * GQ, (kq + 1) * GQ)
    # n1 dequant while n0 matmuls sweep block 1
    for oc in range(4, 6):
        dq_group([(oc, kq) for kq in range(4)])
    chain_mm(1, 0, 0, 0, KT)
    for oc in range(6, 8):
        dq_group([(oc, kq) for kq in range(4)])
    chain_mm(1, 1, 0, 0, KT)
    # n2 dequant under n1 sweep
    for oc in range(8, 10):
        dq_group([(oc, kq) for kq in range(4)])
    block(0, 1)
    for oc in range(10, 12):
        dq_group([(oc, kq) for kq in range(4)])
    block(1, 1)
    # n3 dequant under n2 sweep
    for oc in range(12, 14):
        dq_group([(oc, kq) for kq in range(4)])
    block(0, 2)
    for oc in range(14, 16):
        dq_group([(oc, kq) for kq in range(4)])
    block(1, 2)
    block(0, 3)
    x_unit(2)
    block(1, 3)
    # steady state; xt double-buffer: prefetch sc+1 after the blocks of sc
    for sc in range(2, NSC - 1):
        block_fused(sc)
        if sc + 1 < NSC:
            x_unit(sc + 1)
    for n in range(NCH):
        block(NSC - 1, n)


def build_program(M=M_CORE, N=N_CORE, K=K_FULL):
    nc = bacc.Bacc("TRN2", target_bir_lowering=False, debug=False)
    x = nc.dram_tensor("x", [M, K], F32, kind="ExternalInput")
    q = nc.dram_tensor("qweight", [N, K], I32, kind="ExternalInput")
    s = nc.dram_tensor("scales", [N, K // P], F32, kind="ExternalInput")
    b = nc.dram_tensor("bias", [N], F32, kind="ExternalInput")
    o = nc.dram_tensor("out", [M, N], F32, kind="ExternalOutput")
    with tile.TileContext(nc) as tc:
        with ExitStack() as ctx:
            emit(tc, ctx, o.ap(), x.ap(), q.ap(), s.ap(), b.ap())
    nc.compile()
    return nc


def enable_ntff_profiling():
    """Register the axon NTFF profile hook (the image's antenv lacks
    axon_hooks, so trn_boot degrades silently).  Returns True on success."""
    import sys
    import types

    try:
        from antenv.axon_hooks import get_axon_ntff_profile_hook  # noqa: F401

        return True
    except ImportError:
        pass
    try:
        from trn_agent_boot.trn_boot import _ntff_profile_via_ctypes

        hook = _ntff_profile_via_ctypes("/opt/axon/libaxon_pjrt.so")
        if hook is None:
            return False
        mod = types.ModuleType("antenv.axon_hooks")
        mod._hook = hook

        def set_axon_ntff_profile_hook(h):
            mod._hook = h

        def get_axon_ntff_profile_hook():
            return mod._hook

        mod.set_axon_ntff_profile_hook = set_axon_ntff_profile_hook
        mod.get_axon_ntff_profile_hook = get_axon_ntff_profile_hook
        sys.modules["antenv.axon_hooks"] = mod
        return True
    except Exception:
        return False


_CACHE = {}


def _get_program():
    if "nc" not in _CACHE:
        _CACHE["nc"] = build_program()
    return _CACHE["nc"]


def _shard_inputs(x, qweight, scales, bias):
    x2 = np.asarray(x, dtype=np.float32).reshape(B * S, K_FULL)
    qweight = np.asarray(qweight, dtype=np.int32)
    scales = np.asarray(scales, dtype=np.float32)
    bias = np.asarray(bias, dtype=np.float32)
    in_maps = []
    for c in range(N_CORES):
        mb, nb = divmod(c, NB_SHARDS)
        in_maps.append(
            {
                "x": np.ascontiguousarray(x2[mb * M_CORE : (mb + 1) * M_CORE]),
                "qweight": np.ascontiguousarray(
                    qweight[nb * N_CORE : (nb + 1) * N_CORE]
                ),
                "scales": np.ascontiguousarray(
                    scales[nb * N_CORE : (nb + 1) * N_CORE]
                ),
                "bias": np.ascontiguousarray(bias[nb * N_CORE : (nb + 1) * N_CORE]),
            }
        )
    return in_maps


def _gather_output(results):
    out = np.empty((B * S, NF), dtype=np.float32)
    for c in range(N_CORES):
        mb, nb = divmod(c, NB_SHARDS)
        out[mb * M_CORE : (mb + 1) * M_CORE, nb * N_CORE : (nb + 1) * N_CORE] = (
            results[c]["out"]
        )
    return out.reshape(B, S, NF)


def run_sharded(x, qweight, scales, bias, **spmd_kwargs):
    """Run on all 8 cores; returns (full_output, BassKernelResults)."""
    if spmd_kwargs.get("trace"):
        enable_ntff_profiling()
    nc = _get_program()
    in_maps = _shard_inputs(x, qweight, scales, bias)
    res = bass_utils.run_bass_kernel_spmd(
        nc, in_maps, core_ids=list(range(N_CORES)), **spmd_kwargs
    )
    return _gather_output(res.results), res


def kernel(x, qweight, scales, bias):
    out, _ = run_sharded(x, qweight, scales, bias)
    return out


# revision 18
# speedup vs baseline: 1.0472x; 1.0472x over previous
"""GPTQ group-quantized linear (nn_GPTQLinear) on 8 Trainium2 NeuronCores.

out[b,s,o] = sum_k x[b,s,k] * (qweight[o,k] * scales[o, k//128]) + bias[o]

Full inputs in, full output out.  Sharding (internal): 4-way over batch rows
x 2-way over out_features -> per core M=2048 rows, N=2048 out feats, K=4096.

Per-core kernel (v2 -- PE stream is pure matmuls):
  - qweight int32 [o,k] -> DVE dequant (x group scale bcast along free) ->
    bf16 [o,k] -> SBUF->SBUF DMA-xbar transpose -> resident wT
    [128, K/128, N] bf16 in SBUF.  No PE transposes, no DRAM staging.
  - x fp32 [m,k] -> SWDGE cast DMA (DRAM->SBUF, big row descriptors) ->
    bf16 [m,k] -> SBUF->SBUF DMA-xbar transpose -> xT per 128-row chunk.
  - bf16 matmuls (lhsT = xT slice, rhs = wT slice), fp32 accumulate in PSUM
    over K, bias added by DVE in the epilogue.
  - Startup: first two output chains run in two K-sessions so matmuls start
    after only 1/8th of qweight is dequantized; first 5 x-chunks are swept
    n-group by n-group as dequant completes, so the PE never idles long.
"""

from contextlib import ExitStack

import numpy as np

import concourse.bass as bass
import concourse.bacc as bacc
import concourse.mybir as mybir
import concourse.tile as tile
from concourse import bass_utils
from concourse.masks import make_identity

F32 = mybir.dt.float32
BF16 = mybir.dt.bfloat16
I32 = mybir.dt.int32

P = 128            # partitions = k-tile = quant group size
N_CH = 512         # out-feature chunk (one PSUM bank of fp32)
M_SC = 256         # x rows per block (2 lhsT tiles)

# full problem / sharding constants (hardcoded per harness contract)
B, S, K_FULL, NF = 4, 2048, 4096, 4096
MB_SHARDS, NB_SHARDS = 4, 2
M_CORE, N_CORE = (B * S) // MB_SHARDS, NF // NB_SHARDS
N_CORES = 8


def emit(tc, ctx, o_ap, x_ap, q_ap, s_ap, b_ap):
    nc = tc.nc
    M, K = x_ap.shape
    N = q_ap.shape[0]
    KT = K // P                   # 32 k-tiles
    NCH = N // N_CH               # 4 out-feature chunks
    NSC = M // M_SC               # 8 row blocks of 256
    NOC = N // P                  # 16 o-tiles
    OC_PER_CH = N_CH // P         # 4 o-tiles per out chunk
    MT = M_SC // P                # 2 m-tiles per row block
    KH = K // 2                   # x-cast chunk extent (2048)
    KQ = K // 4                   # dequant quarter-slab extent (1024)
    GQ = KQ // P                  # k-tiles (= groups) per quarter slab (8)

    const = ctx.enter_context(tc.tile_pool(name="const", bufs=1))
    wt_pool = ctx.enter_context(tc.tile_pool(name="wt", bufs=1))
    q_pool = ctx.enter_context(tc.tile_pool(name="qs", bufs=3))
    wdq_pool = ctx.enter_context(tc.tile_pool(name="wdq", bufs=4))
    xbf_pool = ctx.enter_context(tc.tile_pool(name="xbf", bufs=2))
    xt_pool = ctx.enter_context(tc.tile_pool(name="xt", bufs=2))
    out_pool = ctx.enter_context(tc.tile_pool(name="outp", bufs=3))
    ps_pool = ctx.enter_context(tc.tile_pool(name="psmm", bufs=5, space="PSUM"))
    pst_pool = ctx.enter_context(tc.tile_pool(name="pst", bufs=2, space="PSUM"))
    pan_pool = ctx.enter_context(tc.tile_pool(name="pan", bufs=NSC, space="DRAM"))

    # ---- constants ----
    scales_sb = const.tile([P, NOC, KT], F32, tag="scales")
    nc.scalar.dma_start(scales_sb[:], s_ap.rearrange("(oc p) g -> p oc g", p=P))
    ones = const.tile([1, P], BF16, tag="ones")
    nc.vector.memset(ones[:], 1.0)
    identity = const.tile([P, P], BF16, tag="identity")
    make_identity(nc, identity[:])

    # bias row -> partition 0 of bias_bc, then K=1 matmul broadcast in place
    bias_bc = const.tile([P, N], BF16, tag="bias_bc")
    nc.gpsimd.dma_start(bias_bc[0:1, :], b_ap[None, :])
    for n in range(NCH):
        psb = ps_pool.tile([P, N_CH], F32, name="ps_bias", bufs=1)
        nc.tensor.matmul(
            psb[:], ones[:], bias_bc[0:1, n * N_CH : (n + 1) * N_CH],
            start=True, stop=True,
        )
        nc.vector.tensor_copy(bias_bc[:, n * N_CH : (n + 1) * N_CH], psb[:])

    # resident transposed weights: wt[n][p, kt, o] for out chunk n
    wt = [
        wt_pool.tile([P, KT, N_CH], BF16, tag=f"wt{n}", name=f"wt{n}")
        for n in range(NCH)
    ]

    def dq_load_mult(oc, kq):
        """Dequant o-tile oc k-quarter kq: q load + DVE mult into a wdq buf."""
        qt = q_pool.tile([P, KQ], I32, name="qt")
        nc.sync.dma_start(qt[:], q_ap[oc * P : (oc + 1) * P, kq * KQ : (kq + 1) * KQ])
        wdq = wdq_pool.tile([P, KQ], BF16, name="wdq")
        nc.vector.tensor_tensor(
            wdq[:].rearrange("p (g i) -> p g i", i=P),
            qt[:].rearrange("p (g i) -> p g i", i=P),
            scales_sb[:, oc, kq * GQ : (kq + 1) * GQ, None].to_broadcast([P, GQ, P]),
            mybir.AluOpType.mult,
        )
        return wdq

    def dq_transpose_copy(oc, kq, wdq):
        """PE transpose (8 k-tiles through one PSUM bank) -> DVE copy to wt."""
        n_ch, oci = divmod(oc, OC_PER_CH)
        pst = pst_pool.tile([P, KQ], BF16, name="pst")
        for j in range(GQ):
            nc.tensor.transpose(
                pst[:, j * P : (j + 1) * P], wdq[:, j * P : (j + 1) * P], identity[:]
            )
        nc.scalar.activation(
            wt[n_ch][:, kq * GQ : (kq + 1) * GQ, oci * P : (oci + 1) * P],
            pst[:].rearrange("p (g i) -> p g i", i=P),
            mybir.ActivationFunctionType.Copy,
        )

    def dq_group(units):
        """Pipeline a group of dequant units: all mults ahead of all copies."""
        wdqs = [dq_load_mult(oc, kq) for oc, kq in units]
        for (oc, kq), wdq in zip(units, wdqs):
            dq_transpose_copy(oc, kq, wdq)

    xt_tiles = {}

    def x_unit(sc):
        """x rows [sc*256,(sc+1)*256): SWDGE cast to bf16 SBUF, HWDGE stage to
        a DRAM panel, one folded xbar transpose panel -> xT[p, kt, m]."""
        pan = pan_pool.tile([M_SC, K], BF16, name=f"pan{sc}", tag=f"pan{sc}")
        for mt in range(MT):
            m0 = sc * M_SC + mt * P
            for h in range(2):
                xbf = xbf_pool.tile([P, KH], BF16, name="xbf")
                nc.gpsimd.dma_start(xbf[:], x_ap[m0 : m0 + P, h * KH : (h + 1) * KH])
                nc.sync.dma_start(
                    pan[mt * P : (mt + 1) * P, h * KH : (h + 1) * KH], xbf[:]
                )
        xt = xt_pool.tile([P, KT, M_SC], BF16, name="xt")
        nc.sync.dma_start(xt[:], pan[:], transpose=True)
        xt_tiles[sc] = xt
        return xt

    open_chains = {}

    def chain_mm(sc, mt, n, k_lo, k_hi):
        """Emit matmuls k in [k_lo, k_hi) of chain (sc, mt, n); close at K."""
        if k_lo == 0:
            open_chains[(sc, mt, n)] = ps_pool.tile([P, N_CH], F32, name="ps_mm")
        ps = open_chains[(sc, mt, n)]
        xt = xt_tiles[sc]
        for k in range(k_lo, k_hi):
            nc.tensor.matmul(
                ps[:], xt[:, k, mt * P : (mt + 1) * P], wt[n][:, k, :],
                start=(k == 0), stop=(k == KT - 1),
            )
        if k_hi < KT:
            return
        del open_chains[(sc, mt, n)]
        ot = out_pool.tile([P, N_CH], F32, name="ot")
        nc.vector.tensor_tensor(
            ot[:], ps[:], bias_bc[:, n * N_CH : (n + 1) * N_CH],
            mybir.AluOpType.add,
        )
        m0 = sc * M_SC + mt * P
        nc.sync.dma_start(o_ap[m0 : m0 + P, n * N_CH : (n + 1) * N_CH], ot[:])

    def block(sc, n):
        for mt in range(MT):
            chain_mm(sc, mt, n, 0, KT)

    def block_fused(sc):
        """All 4 out chunks of one m-tile per k step: stationary xt tile is
        reused across the 4 rhs chunks."""
        xt = xt_tiles[sc]
        for mt in range(MT):
            pss = [ps_pool.tile([P, N_CH], F32, name="ps_mm") for _ in range(NCH)]
            for k in range(KT):
                for n in range(NCH):
                    nc.tensor.matmul(
                        pss[n][:], xt[:, k, mt * P : (mt + 1) * P], wt[n][:, k, :],
                        start=(k == 0), stop=(k == KT - 1),
                    )
            m0 = sc * M_SC + mt * P
            for n in range(NCH):
                ot = out_pool.tile([P, N_CH], F32, name="ot")
                nc.vector.tensor_tensor(
                    ot[:], pss[n][:], bias_bc[:, n * N_CH : (n + 1) * N_CH],
                    mybir.AluOpType.add,
                )
                nc.sync.dma_start(
                    o_ap[m0 : m0 + P, n * N_CH : (n + 1) * N_CH], ot[:]
                )

    # ---- schedule ----
    x_unit(0)
    x_unit(1)
    # n0 quarter-by-quarter; block-0 chains run in 4 K-sessions
    for kq in range(4):
        dq_group([(oc, kq) for oc in range(0, 4)])
        chain_mm(0, 0, 0, kq * GQ, (kq + 1) * GQ)
        chain_mm(0, 1, 0, kq * GQ, (kq + 1) * GQ)
    # n1 dequant while n0 matmuls sweep block 1
    for oc in range(4, 6):
        dq_group([(oc, kq) for kq in range(4)])
    chain_mm(1, 0, 0, 0, KT)
    for oc in range(6, 8):
        dq_group([(oc, kq) for kq in range(4)])
    chain_mm(1, 1, 0, 0, KT)
    # n2 dequant under n1 sweep
    for oc in range(8, 10):
        dq_group([(oc, kq) for kq in range(4)])
    block(0, 1)
    for oc in range(10, 12):
        dq_group([(oc, kq) for kq in range(4)])
    block(1, 1)
    # n3 dequant under n2 sweep
    for oc in range(12, 14):
        dq_group([(oc, kq) for kq in range(4)])
    block(0, 2)
    for oc in range(14, 16):
        dq_group([(oc, kq) for kq in range(4)])
    block(1, 2)
    block(0, 3)
    x_unit(2)
    block(1, 3)
    # steady state; xt double-buffer: prefetch sc+1 after the blocks of sc
    for sc in range(2, NSC - 1):
        block_fused(sc)
        if sc + 1 < NSC:
            x_unit(sc + 1)
    for n in range(NCH):
        block(NSC - 1, n)


def build_program(M=M_CORE, N=N_CORE, K=K_FULL):
    nc = bacc.Bacc("TRN2", target_bir_lowering=False, debug=False)
    x = nc.dram_tensor("x", [M, K], F32, kind="ExternalInput")
    q = nc.dram_tensor("qweight", [N, K], I32, kind="ExternalInput")
    s = nc.dram_tensor("scales", [N, K // P], F32, kind="ExternalInput")
    b = nc.dram_tensor("bias", [N], F32, kind="ExternalInput")
    o = nc.dram_tensor("out", [M, N], F32, kind="ExternalOutput")
    with tile.TileContext(nc) as tc:
        with ExitStack() as ctx:
            emit(tc, ctx, o.ap(), x.ap(), q.ap(), s.ap(), b.ap())
    nc.compile()
    return nc


def enable_ntff_profiling():
    """Register the axon NTFF profile hook (the image's antenv lacks
    axon_hooks, so trn_boot degrades silently).  Returns True on success."""
    import sys
    import types

    try:
        from antenv.axon_hooks import get_axon_ntff_profile_hook  # noqa: F401

        return True
    except ImportError:
        pass
    try:
        from trn_agent_boot.trn_boot import _ntff_profile_via_ctypes

        hook = _ntff_profile_via_ctypes("/opt/axon/libaxon_pjrt.so")
        if hook is None:
            return False
        mod = types.ModuleType("antenv.axon_hooks")
        mod._hook = hook

        def set_axon_ntff_profile_hook(h):
            mod._hook = h

        def get_axon_ntff_profile_hook():
            return mod._hook

        mod.set_axon_ntff_profile_hook = set_axon_ntff_profile_hook
        mod.get_axon_ntff_profile_hook = get_axon_ntff_profile_hook
        sys.modules["antenv.axon_hooks"] = mod
        return True
    except Exception:
        return False


_CACHE = {}


def _get_program():
    if "nc" not in _CACHE:
        _CACHE["nc"] = build_program()
    return _CACHE["nc"]


def _shard_inputs(x, qweight, scales, bias):
    x2 = np.asarray(x, dtype=np.float32).reshape(B * S, K_FULL)
    qweight = np.asarray(qweight, dtype=np.int32)
    scales = np.asarray(scales, dtype=np.float32)
    bias = np.asarray(bias, dtype=np.float32)
    in_maps = []
    for c in range(N_CORES):
        mb, nb = divmod(c, NB_SHARDS)
        in_maps.append(
            {
                "x": np.ascontiguousarray(x2[mb * M_CORE : (mb + 1) * M_CORE]),
                "qweight": np.ascontiguousarray(
                    qweight[nb * N_CORE : (nb + 1) * N_CORE]
                ),
                "scales": np.ascontiguousarray(
                    scales[nb * N_CORE : (nb + 1) * N_CORE]
                ),
                "bias": np.ascontiguousarray(bias[nb * N_CORE : (nb + 1) * N_CORE]),
            }
        )
    return in_maps


def _gather_output(results):
    out = np.empty((B * S, NF), dtype=np.float32)
    for c in range(N_CORES):
        mb, nb = divmod(c, NB_SHARDS)
        out[mb * M_CORE : (mb + 1) * M_CORE, nb * N_CORE : (nb + 1) * N_CORE] = (
            results[c]["out"]
        )
    return out.reshape(B, S, NF)


def run_sharded(x, qweight, scales, bias, **spmd_kwargs):
    """Run on all 8 cores; returns (full_output, BassKernelResults)."""
    if spmd_kwargs.get("trace"):
        enable_ntff_profiling()
    nc = _get_program()
    in_maps = _shard_inputs(x, qweight, scales, bias)
    res = bass_utils.run_bass_kernel_spmd(
        nc, in_maps, core_ids=list(range(N_CORES)), **spmd_kwargs
    )
    return _gather_output(res.results), res


def kernel(x, qweight, scales, bias):
    out, _ = run_sharded(x, qweight, scales, bias)
    return out
